# revision 24
# baseline (speedup 1.0000x reference)
"""Trainium2 Bass kernel for MMoE (3 tasks, 16 experts, top-4 gating).

Strategy: data-parallel over the batch (512 tokens/core) with host-side
top-k routing and device-side SPARSE expert compute in bf16.

Host: computes gating logits/top-4/softmax in fp32 (0.07% of total FLOPs),
builds per-(core, expert) compacted token lists, gathers x rows into
compacted per-expert blocks, and emits 0/1 scatter matrices that map
compacted fc2 outputs back to dense token order.

Device (per core):
 - fc1 runs weight-stationary on the compacted token stream: cost scales
   with the actual number of routed tokens (~60% of dense).
 - fc2 consumes hT (already in lhsT layout) per 128-token compact tile.
 - exp(out) on ScalarE; a small 0/1-matrix matmul scatters each expert's
   exp(out) tile into dense token order in PSUM; VectorE then applies the
   host-computed fp32 gates with fused MACs into the per-task combine
   accumulator. Log + store as before.

Unselected (expert, token) pairs have gate == 0 in every task, so skipping
them reproduces the reference's dense-masked math exactly (bf16 rounding
aside). Capacities/spans are baked into the compiled graph from the first
call's routing and the graph is rebuilt if a later call overflows them.
"""
import numpy as np
import ml_dtypes

import concourse.mybir as mybir
import concourse.tile as tile
from concourse import bacc
from concourse.bass_utils import run_bass_kernel_spmd

F32 = mybir.dt.float32
BF16 = mybir.dt.bfloat16
AF = mybir.ActivationFunctionType
ALU = mybir.AluOpType
AX = mybir.AxisListType
BF = ml_dtypes.bfloat16

T, B, IN, HID, OUT, E, TOPK = 3, 4096, 1024, 2048, 1024, 16, 4
NCORES = 8
P = 128
BSH = B // NCORES          # tokens per core
NBT = BSH // P             # dense token tiles per core
NIC = IN // P              # contraction tiles for fc1
NJT = HID // P             # hid tiles
NQ = 4                     # fc1 weight stream granularity (quarters)
JQ = NJT // NQ
JH = NJT // 2              # j-chunks per fc2 half


def _route(x, w_gate):
    """Host fp32 gating identical to the reference's math."""
    logits = np.einsum('bi,tie->tbe', x.astype(np.float32),
                       w_gate.astype(np.float32))
    idx = np.argsort(-logits, axis=-1, kind='stable')[..., :TOPK]
    top = np.take_along_axis(logits, idx, -1)
    g = np.exp(top - top.max(-1, keepdims=True))
    g = (g / g.sum(-1, keepdims=True)).astype(np.float32)
    gates = np.zeros((T, B, E), np.float32)
    np.put_along_axis(gates, idx, g, -1)
    gates = np.where(gates <= 1e-4, np.float32(0.0), gates)
    return gates                                 # [T, B, E] fp32


def _plan(gates):
    """Static structure: capacities, tile counts, scatter-block spans."""
    mask = (gates > 0).any(0)                    # [B, E]
    lists = [[None] * E for _ in range(NCORES)]
    cnt = np.zeros((NCORES, E), int)
    for c in range(NCORES):
        sub = mask[c * BSH:(c + 1) * BSH]
        for e in range(E):
            sel = np.nonzero(sub[:, e])[0].astype(np.int32)
            lists[c][e] = sel
            cnt[c, e] = len(sel)
    ncap = cnt.max(0)
    ncap = ((ncap + 3) // 4) * 4                 # DMA-friendly alignment
    ncap = np.minimum(ncap, BSH)
    nct = (ncap + P - 1) // P
    # dt span per (e, ct): union over cores of the dense tile range covered
    spans = []
    for e in range(E):
        es = []
        for ct in range(nct[e]):
            lo, hi = NBT, -1
            for c in range(NCORES):
                sl = lists[c][e][ct * P:(ct + 1) * P]
                if len(sl):
                    lo = min(lo, int(sl[0]) // P)
                    hi = max(hi, int(sl[-1]) // P)
            es.append((lo, hi) if hi >= lo else None)
        spans.append(es)
    return mask, lists, ncap, nct, spans


class MMoEKernel:
    def __init__(self, ncap, nct, spans):
        self.ncap = [int(v) for v in ncap]
        self.nct = [int(v) for v in nct]
        self.spans = spans
        self.sumcap = int(sum(self.ncap))
        self.off = np.concatenate([[0], np.cumsum(self.ncap)]).astype(int)
        # flatten scatter blocks: (e, ct, dt) -> block index
        self.blkidx = {}
        nb = 0
        for e in range(E):
            for ct in range(self.nct[e]):
                sp = spans[e][ct]
                if sp is None:
                    continue
                for dt in range(sp[0], sp[1] + 1):
                    self.blkidx[(e, ct, dt)] = nb
                    nb += 1
        self.nblk = nb
        # per (e, dt): list of ct contributing
        self.dtmap = [[[] for _ in range(NBT)] for _ in range(E)]
        for (e, ct, dt), b in self.blkidx.items():
            self.dtmap[e][dt].append((ct, b))
        self.nc = None

    # ---------------- device graph ----------------
    def build(self):
        ncap, nct = self.ncap, self.nct
        nc = bacc.Bacc(None, target_bir_lowering=False, debug=False)
        xce = [nc.declare_dram_parameter(
            f"xc{e}", [P, NIC, self.ncap[e]], BF16, isOutput=False)
            for e in range(E)]
        w1t = nc.declare_dram_parameter(
            "w1t", [E, NQ, P, NIC, HID // NQ], BF16, isOutput=False)
        w2t = nc.declare_dram_parameter(
            "w2t", [E, 2, P, JH, OUT], BF16, isOutput=False)
        b1t = nc.declare_dram_parameter("b1t", [P, E * NJT], F32,
                                        isOutput=False)
        scat = nc.declare_dram_parameter(
            "scat", [P, max(self.nblk, 1), P], BF16, isOutput=False)
        gt = nc.declare_dram_parameter("gt", [P, NBT, T * E], F32,
                                       isOutput=False)
        out_ext = nc.declare_dram_parameter(
            "out", [T, BSH, OUT], F32, isOutput=True)

        with tile.TileContext(nc) as tc:
            import contextlib
            with contextlib.ExitStack() as ctx:
                const = ctx.enter_context(tc.tile_pool(name="const", bufs=1))
                xe_p = ctx.enter_context(tc.tile_pool(name="xe", bufs=2))
                w1_p = ctx.enter_context(tc.tile_pool(name="w1", bufs=2))
                w2_p = ctx.enter_context(tc.tile_pool(name="w2", bufs=2))
                sb_p = ctx.enter_context(tc.tile_pool(name="sb", bufs=2))
                h_p = ctx.enter_context(tc.tile_pool(name="h", bufs=2))
                eg_p = ctx.enter_context(tc.tile_pool(name="eg", bufs=2))
                comb_p = ctx.enter_context(tc.tile_pool(name="comb", bufs=1))
                ph_p = ctx.enter_context(
                    tc.tile_pool(name="ph", bufs=2, space="PSUM"))
                po_p = ctx.enter_context(
                    tc.tile_pool(name="po", bufs=2, space="PSUM"))
                dg_p = ctx.enter_context(
                    tc.tile_pool(name="dg", bufs=2, space="PSUM"))

                # resident inputs; critical-path DMAs first.
                pre_xe = xe_p.tile([P, NIC, ncap[0]], BF16, tag="xe")
                nc.sync.dma_start(out=pre_xe[:], in_=xce[0][:, :, :])
                pre_w1 = w1_p.tile([P, NIC, HID // NQ], BF16, tag="w1sb")
                nc.scalar.dma_start(out=pre_w1[:], in_=w1t[0, 0, :, :, :])
                b1sb = const.tile([P, E * NJT], F32)
                nc.scalar.dma_start(out=b1sb[:], in_=b1t[:, :])
                gsb = const.tile([P, NBT, T * E], F32)
                nc.gpsimd.dma_start(out=gsb[:], in_=gt[:, :, :])
                pre_w2 = []
                for hh in range(2):
                    w2sb = w2_p.tile([P, JH, OUT], BF16, tag=f"w2h{hh}")
                    nc.gpsimd.dma_start(out=w2sb[:], in_=w2t[0, hh, :, :, :])
                    pre_w2.append(w2sb)
                comb = comb_p.tile([P, T * NBT, OUT], F32)
                nc.vector.memset(comb[:], 0.0)

                # ---------------- expert loop ----------------
                for e in range(E):
                    ncp, nt_e = ncap[e], nct[e]
                    ncp128 = nt_e * P
                    blks = sorted({b for dt in range(NBT)
                                   for (_, b) in self.dtmap[e][dt]})
                    b0, nbe = (blks[0], len(blks)) if blks else (0, 0)
                    if e == 0:
                        xe, w2h = pre_xe, pre_w2
                    else:
                        xe = xe_p.tile([P, NIC, ncp], BF16, tag="xe")
                        nc.sync.dma_start(out=xe[:], in_=xce[e][:, :, :])
                        w2h = []
                        for hh in range(2):
                            w2sb = w2_p.tile([P, JH, OUT], BF16,
                                             tag=f"w2h{hh}")
                            nc.sync.dma_start(out=w2sb[:],
                                              in_=w2t[e, hh, :, :, :])
                            w2h.append(w2sb)
                    sblk = sb_p.tile([P, max(nbe, 1), P], BF16, tag="sblk")
                    if nbe:
                        nc.gpsimd.dma_start(out=sblk[:],
                                            in_=scat[:, b0:b0 + nbe, :])

                    hT = h_p.tile([P, NJT, ncp128], BF16, tag="hT")
                    if ncp128 > ncp:
                        nc.vector.memset(hT[:, :, ncp:ncp128], 0.0)
                    w1sb = None
                    for jt in range(NJT):
                        q, jj = divmod(jt, JQ)
                        if jj == 0:
                            if e == 0 and q == 0:
                                w1sb = pre_w1
                            else:
                                w1sb = w1_p.tile([P, NIC, HID // NQ], BF16,
                                                 tag="w1sb")
                                nc.sync.dma_start(out=w1sb[:],
                                                  in_=w1t[e, q, :, :, :])
                        ph = ph_p.tile([P, ncp], F32)
                        for ic in range(NIC):
                            nc.tensor.matmul(
                                ph[:], lhsT=w1sb[:, ic, jj * P:(jj + 1) * P],
                                rhs=xe[:, ic, :],
                                start=(ic == 0), stop=(ic == NIC - 1))
                        nc.scalar.activation(
                            hT[:, jt, 0:ncp], ph[:], AF.Relu,
                            bias=b1sb[:, e * NJT + jt: e * NJT + jt + 1])

                    def emit_scatter(dt, cts):
                        for oh in range(2):
                            dg = dg_p.tile([P, 512], F32)
                            for i, (ct, b) in enumerate(cts):
                                nc.tensor.matmul(
                                    dg[:], lhsT=sblk[:, b - b0, :],
                                    rhs=egc[:, ct, oh * 512:(oh + 1) * 512],
                                    start=(i == 0), stop=(i == len(cts) - 1))
                            for t in range(T):
                                gcol = gsb[:, dt, t * E + e: t * E + e + 1]
                                dst = comb[:, t * NBT + dt,
                                           oh * 512:(oh + 1) * 512]
                                nc.vector.scalar_tensor_tensor(
                                    dst, dg[:], gcol, dst,
                                    op0=ALU.mult, op1=ALU.add)

                    ready = {dt: max(c for c, _ in self.dtmap[e][dt])
                             for dt in range(NBT) if self.dtmap[e][dt]}
                    egc = eg_p.tile([P, nt_e, OUT], BF16, tag="egc")
                    for ct in range(nt_e):
                        po = po_p.tile([P, OUT], F32)
                        for jc in range(NJT):
                            hh, jj = divmod(jc, JH)
                            for oh in range(2):
                                nc.tensor.matmul(
                                    po[:, oh * 512:(oh + 1) * 512],
                                    lhsT=hT[:, jc, ct * P:(ct + 1) * P],
                                    rhs=w2h[hh][:, jj, oh * 512:(oh + 1) * 512],
                                    start=(jc == 0), stop=(jc == NJT - 1))
                        nc.scalar.activation(egc[:, ct, :], po[:], AF.Exp)
                        for dt in sorted(ready):
                            if ready[dt] == ct:
                                emit_scatter(dt, self.dtmap[e][dt])

                # ---------------- log + output ----------------
                # dense-tile-major: tile bt's combine finishes before bt+1,
                # so its 3 tasks drain first; outputs split across queues.
                for bt in range(NBT):
                    for t in range(T):
                        cslice = comb[:, t * NBT + bt, :]
                        nc.scalar.activation(cslice, cslice, AF.Ln)
                        q = nc.sync if (bt * T + t) % 2 == 0 else nc.scalar
                        q.dma_start(
                            out=out_ext[t, bt * P:(bt + 1) * P, :], in_=cslice)

        nc.compile()
        self.nc = nc
        return nc

    # ---------------- host-side marshalling ----------------
    def marshal_shared(self, fc1_w, fc1_b, fc2_w):
        w1t = np.empty((E, NQ, P, NIC, HID // NQ), dtype=BF)
        w2t = np.empty((E, 2, P, JH, OUT), dtype=BF)
        for e in range(E):
            a = fc1_w[e].T.reshape(NIC, P, HID).transpose(1, 0, 2)
            for q in range(NQ):
                w1t[e, q] = a[:, :, q * (HID // NQ):(q + 1) * (HID // NQ)]
            bm = fc2_w[e].T.reshape(NJT, P, OUT).transpose(1, 0, 2)
            for hh in range(2):
                w2t[e, hh] = bm[:, hh * JH:(hh + 1) * JH, :]
        b1t = np.ascontiguousarray(
            fc1_b.reshape(E, NJT, P).transpose(2, 0, 1)
            .reshape(P, E * NJT)).astype(np.float32)
        return dict(w1t=w1t, w2t=w2t, b1t=b1t)

    def marshal_core(self, c, x, gates, lists):
        """Per-core inputs: compacted x, scatter blocks, gates. Returns
        None if this core's routing does not fit the compiled plan."""
        xt = np.ascontiguousarray(
            x[c * BSH:(c + 1) * BSH].T.reshape(NIC, P, BSH)
            .transpose(1, 0, 2)).astype(BF)       # [P, NIC, BSH]
        m = {}
        scat = np.zeros((P, max(self.nblk, 1), P), BF)
        for e in range(E):
            sel = lists[c][e]
            n = len(sel)
            if n > self.ncap[e]:
                return None, None
            idx = np.zeros(self.ncap[e], np.int64)
            idx[:n] = sel
            m[f"xc{e}"] = np.ascontiguousarray(xt[:, :, idx])
            for ct in range((n + P - 1) // P):
                sl = sel[ct * P:(ct + 1) * P]
                for s, tok in enumerate(sl):
                    dt, r = divmod(int(tok), P)
                    b = self.blkidx.get((e, ct, dt))
                    if b is None:
                        return None, None
                    scat[s, b, r] = 1
        gtile = np.ascontiguousarray(
            gates[:, c * BSH:(c + 1) * BSH, :]      # [T, BSH, E]
            .transpose(1, 0, 2).reshape(NBT, P, T * E)
            .transpose(1, 0, 2)).astype(np.float32)
        m.update(scat=scat, gt=gtile)
        return m, True

    def run(self, x, w_gate, fc1_w, fc1_b, fc2_w, fc2_b, _prep=None):
        if _prep is None:
            gates = _route(x, w_gate)
            _, lists, _, _, _ = _plan(gates)
        else:
            gates, lists = _prep
        if self.nc is None:
            self.build()
        shared = self.marshal_shared(fc1_w, fc1_b, fc2_w)
        in_maps = []
        for c in range(NCORES):
            m, ok = self.marshal_core(c, x, gates, lists)
            if not ok:
                return None, None
            m.update(shared)
            in_maps.append(m)
        res = run_bass_kernel_spmd(self.nc, in_maps,
                                   core_ids=list(range(NCORES)))
        out = np.concatenate(
            [res.results[c]["out"] for c in range(NCORES)], axis=1)
        return np.ascontiguousarray(out.astype(np.float32)), res


_KERNEL = None


def kernel(x, w_gate, fc1_w, fc1_b, fc2_w, fc2_b):
    global _KERNEL
    x = np.asarray(x, dtype=np.float32)
    w_gate = np.asarray(w_gate, dtype=np.float32)
    fc1_w = np.asarray(fc1_w, dtype=np.float32)
    fc1_b = np.asarray(fc1_b, dtype=np.float32)
    fc2_w = np.asarray(fc2_w, dtype=np.float32)
    fc2_b = np.asarray(fc2_b, dtype=np.float32)
    assert np.all(fc2_b == 0), "fc2 bias unsupported in sparse path"
    gates = _route(x, w_gate)
    _, lists, ncap, nct, spans = _plan(gates)
    if _KERNEL is None:
        _KERNEL = MMoEKernel(ncap, nct, spans)
    out, _ = _KERNEL.run(x, w_gate, fc1_w, fc1_b, fc2_w, fc2_b,
                         _prep=(gates, lists))
    if out is None:            # routing overflowed the compiled plan
        _KERNEL = MMoEKernel(ncap, nct, spans)
        out, _ = _KERNEL.run(x, w_gate, fc1_w, fc1_b, fc2_w, fc2_b,
                             _prep=(gates, lists))
        assert out is not None
    return out


# revision 25
# speedup vs baseline: 1.0238x; 1.0238x over previous
"""Trainium2 Bass kernel for MMoE (3 tasks, 16 experts, top-4 gating).

Strategy: data-parallel over the batch (512 tokens/core) with host-side
top-k routing and device-side SPARSE expert compute in bf16.

Host: computes gating logits/top-4/softmax in fp32 (0.07% of total FLOPs),
builds per-(core, expert) compacted token lists, gathers x rows into
compacted per-expert blocks, and emits 0/1 scatter matrices that map
compacted fc2 outputs back to dense token order.

Device (per core):
 - fc1 runs weight-stationary on the compacted token stream: cost scales
   with the actual number of routed tokens (~60% of dense).
 - fc2 consumes hT (already in lhsT layout) per 128-token compact tile.
 - exp(out) on ScalarE; a small 0/1-matrix matmul scatters each expert's
   exp(out) tile into dense token order in PSUM; VectorE then applies the
   host-computed fp32 gates with fused MACs into the per-task combine
   accumulator. Log + store as before.

Unselected (expert, token) pairs have gate == 0 in every task, so skipping
them reproduces the reference's dense-masked math exactly (bf16 rounding
aside). Capacities/spans are baked into the compiled graph from the first
call's routing and the graph is rebuilt if a later call overflows them.
"""
import numpy as np
import ml_dtypes

import concourse.mybir as mybir
import concourse.tile as tile
from concourse import bacc
from concourse.bass_utils import run_bass_kernel_spmd

F32 = mybir.dt.float32
BF16 = mybir.dt.bfloat16
AF = mybir.ActivationFunctionType
ALU = mybir.AluOpType
AX = mybir.AxisListType
BF = ml_dtypes.bfloat16

T, B, IN, HID, OUT, E, TOPK = 3, 4096, 1024, 2048, 1024, 16, 4
NCORES = 8
P = 128
BSH = B // NCORES          # tokens per core
NBT = BSH // P             # dense token tiles per core
NIC = IN // P              # contraction tiles for fc1
NJT = HID // P             # hid tiles
NQ = 4                     # fc1 weight stream granularity (quarters)
JQ = NJT // NQ
JH = NJT // 2              # j-chunks per fc2 half


def _route(x, w_gate):
    """Host fp32 gating identical to the reference's math."""
    logits = np.einsum('bi,tie->tbe', x.astype(np.float32),
                       w_gate.astype(np.float32))
    idx = np.argsort(-logits, axis=-1, kind='stable')[..., :TOPK]
    top = np.take_along_axis(logits, idx, -1)
    g = np.exp(top - top.max(-1, keepdims=True))
    g = (g / g.sum(-1, keepdims=True)).astype(np.float32)
    gates = np.zeros((T, B, E), np.float32)
    np.put_along_axis(gates, idx, g, -1)
    gates = np.where(gates <= 1e-4, np.float32(0.0), gates)
    return gates                                 # [T, B, E] fp32


def _plan(gates):
    """Static structure: capacities, tile counts, scatter-block spans."""
    mask = (gates > 0).any(0)                    # [B, E]
    lists = [[None] * E for _ in range(NCORES)]
    cnt = np.zeros((NCORES, E), int)
    for c in range(NCORES):
        sub = mask[c * BSH:(c + 1) * BSH]
        for e in range(E):
            sel = np.nonzero(sub[:, e])[0].astype(np.int32)
            lists[c][e] = sel
            cnt[c, e] = len(sel)
    ncap = cnt.max(0)
    ncap = ((ncap + 3) // 4) * 4                 # DMA-friendly alignment
    ncap = np.minimum(ncap, BSH)
    nct = (ncap + P - 1) // P
    # dt span per (e, ct): union over cores of the dense tile range covered
    spans = []
    for e in range(E):
        es = []
        for ct in range(nct[e]):
            lo, hi = NBT, -1
            for c in range(NCORES):
                sl = lists[c][e][ct * P:(ct + 1) * P]
                if len(sl):
                    lo = min(lo, int(sl[0]) // P)
                    hi = max(hi, int(sl[-1]) // P)
            es.append((lo, hi) if hi >= lo else None)
        spans.append(es)
    return mask, lists, ncap, nct, spans


class MMoEKernel:
    def __init__(self, ncap, nct, spans):
        self.ncap = [int(v) for v in ncap]
        self.nct = [int(v) for v in nct]
        self.spans = spans
        self.sumcap = int(sum(self.ncap))
        self.off = np.concatenate([[0], np.cumsum(self.ncap)]).astype(int)
        # flatten scatter blocks: (e, ct, dt) -> block index
        self.blkidx = {}
        nb = 0
        for e in range(E):
            for ct in range(self.nct[e]):
                sp = spans[e][ct]
                if sp is None:
                    continue
                for dt in range(sp[0], sp[1] + 1):
                    self.blkidx[(e, ct, dt)] = nb
                    nb += 1
        self.nblk = nb
        # per (e, dt): list of ct contributing
        self.dtmap = [[[] for _ in range(NBT)] for _ in range(E)]
        for (e, ct, dt), b in self.blkidx.items():
            self.dtmap[e][dt].append((ct, b))
        self.nc = None

    # ---------------- device graph ----------------
    def build(self):
        ncap, nct = self.ncap, self.nct
        nc = bacc.Bacc(None, target_bir_lowering=False, debug=False)
        xc = nc.declare_dram_parameter(
            "xc", [P, NIC, self.sumcap], BF16, isOutput=False)
        w1t = nc.declare_dram_parameter(
            "w1t", [E, NQ, P, NIC, HID // NQ], BF16, isOutput=False)
        w2t = nc.declare_dram_parameter(
            "w2t", [E, 2, P, JH, OUT], BF16, isOutput=False)
        b1t = nc.declare_dram_parameter("b1t", [P, E * NJT], F32,
                                        isOutput=False)
        scat = nc.declare_dram_parameter(
            "scat", [P, max(self.nblk, 1), P], BF16, isOutput=False)
        gt = nc.declare_dram_parameter("gt", [P, NBT, T * E], F32,
                                       isOutput=False)
        out_ext = nc.declare_dram_parameter(
            "out", [T, BSH, OUT], F32, isOutput=True)

        with tile.TileContext(nc) as tc:
            import contextlib
            with contextlib.ExitStack() as ctx:
                const = ctx.enter_context(tc.tile_pool(name="const", bufs=1))
                xe_p = ctx.enter_context(tc.tile_pool(name="xe", bufs=2))
                w1_p = ctx.enter_context(tc.tile_pool(name="w1", bufs=2))
                w2_p = ctx.enter_context(tc.tile_pool(name="w2", bufs=2))
                sb_p = ctx.enter_context(tc.tile_pool(name="sb", bufs=2))
                h_p = ctx.enter_context(tc.tile_pool(name="h", bufs=2))
                eg_p = ctx.enter_context(tc.tile_pool(name="eg", bufs=2))
                comb_p = ctx.enter_context(tc.tile_pool(name="comb", bufs=1))
                ph_p = ctx.enter_context(
                    tc.tile_pool(name="ph", bufs=2, space="PSUM"))
                po_p = ctx.enter_context(
                    tc.tile_pool(name="po", bufs=2, space="PSUM"))
                dg_p = ctx.enter_context(
                    tc.tile_pool(name="dg", bufs=2, space="PSUM"))

                # resident inputs; critical-path DMAs first.
                pre_xe = xe_p.tile([P, NIC, ncap[0]], BF16, tag="xe")
                nc.sync.dma_start(out=pre_xe[:], in_=xc[:, :, 0:ncap[0]])
                pre_w1 = w1_p.tile([P, NIC, HID // NQ], BF16, tag="w1sb")
                nc.scalar.dma_start(out=pre_w1[:], in_=w1t[0, 0, :, :, :])
                b1sb = const.tile([P, E * NJT], F32)
                nc.scalar.dma_start(out=b1sb[:], in_=b1t[:, :])
                gsb = const.tile([P, NBT, T * E], F32)
                nc.gpsimd.dma_start(out=gsb[:], in_=gt[:, :, :])
                pre_w2 = []
                for hh in range(2):
                    w2sb = w2_p.tile([P, JH, OUT], BF16, tag=f"w2h{hh}")
                    nc.gpsimd.dma_start(out=w2sb[:], in_=w2t[0, hh, :, :, :])
                    pre_w2.append(w2sb)
                comb = comb_p.tile([P, T * NBT, OUT], F32)
                nc.vector.memset(comb[:], 0.0)

                # ---------------- expert loop ----------------
                for e in range(E):
                    ncp, nt_e = ncap[e], nct[e]
                    ncp128 = nt_e * P
                    blks = sorted({b for dt in range(NBT)
                                   for (_, b) in self.dtmap[e][dt]})
                    b0, nbe = (blks[0], len(blks)) if blks else (0, 0)
                    if e == 0:
                        xe, w2h = pre_xe, pre_w2
                    else:
                        xe = xe_p.tile([P, NIC, ncp], BF16, tag="xe")
                        nc.sync.dma_start(
                            out=xe[:],
                            in_=xc[:, :, self.off[e]:self.off[e] + ncp])
                        w2h = []
                        for hh in range(2):
                            w2sb = w2_p.tile([P, JH, OUT], BF16,
                                             tag=f"w2h{hh}")
                            nc.sync.dma_start(out=w2sb[:],
                                              in_=w2t[e, hh, :, :, :])
                            w2h.append(w2sb)
                    sblk = sb_p.tile([P, max(nbe, 1), P], BF16, tag="sblk")
                    if nbe:
                        nc.gpsimd.dma_start(out=sblk[:],
                                            in_=scat[:, b0:b0 + nbe, :])

                    hT = h_p.tile([P, NJT, ncp128], BF16, tag="hT")
                    if ncp128 > ncp:
                        nc.vector.memset(hT[:, :, ncp:ncp128], 0.0)
                    w1sb = None
                    for jt in range(NJT):
                        q, jj = divmod(jt, JQ)
                        if jj == 0:
                            if e == 0 and q == 0:
                                w1sb = pre_w1
                            else:
                                w1sb = w1_p.tile([P, NIC, HID // NQ], BF16,
                                                 tag="w1sb")
                                nc.sync.dma_start(out=w1sb[:],
                                                  in_=w1t[e, q, :, :, :])
                        ph = ph_p.tile([P, ncp], F32)
                        for ic in range(NIC):
                            nc.tensor.matmul(
                                ph[:], lhsT=w1sb[:, ic, jj * P:(jj + 1) * P],
                                rhs=xe[:, ic, :],
                                start=(ic == 0), stop=(ic == NIC - 1))
                        nc.scalar.activation(
                            hT[:, jt, 0:ncp], ph[:], AF.Relu,
                            bias=b1sb[:, e * NJT + jt: e * NJT + jt + 1])

                    def emit_scatter(dt, cts):
                        for oh in range(2):
                            dg = dg_p.tile([P, 512], F32)
                            for i, (ct, b) in enumerate(cts):
                                nc.tensor.matmul(
                                    dg[:], lhsT=sblk[:, b - b0, :],
                                    rhs=egc[:, ct, oh * 512:(oh + 1) * 512],
                                    start=(i == 0), stop=(i == len(cts) - 1))
                            for t in range(T):
                                gcol = gsb[:, dt, t * E + e: t * E + e + 1]
                                dst = comb[:, t * NBT + dt,
                                           oh * 512:(oh + 1) * 512]
                                nc.vector.scalar_tensor_tensor(
                                    dst, dg[:], gcol, dst,
                                    op0=ALU.mult, op1=ALU.add)

                    ready = {dt: max(c for c, _ in self.dtmap[e][dt])
                             for dt in range(NBT) if self.dtmap[e][dt]}
                    egc = eg_p.tile([P, nt_e, OUT], BF16, tag="egc")
                    for ct in range(nt_e):
                        po = po_p.tile([P, OUT], F32)
                        for jc in range(NJT):
                            hh, jj = divmod(jc, JH)
                            for oh in range(2):
                                nc.tensor.matmul(
                                    po[:, oh * 512:(oh + 1) * 512],
                                    lhsT=hT[:, jc, ct * P:(ct + 1) * P],
                                    rhs=w2h[hh][:, jj, oh * 512:(oh + 1) * 512],
                                    start=(jc == 0), stop=(jc == NJT - 1))
                        nc.scalar.activation(egc[:, ct, :], po[:], AF.Exp)
                        for dt in sorted(ready):
                            if ready[dt] == ct:
                                emit_scatter(dt, self.dtmap[e][dt])

                # ---------------- log + output ----------------
                for t in range(T):
                    for bt in range(NBT):
                        cslice = comb[:, t * NBT + bt, :]
                        nc.scalar.activation(cslice, cslice, AF.Ln)
                        nc.sync.dma_start(
                            out=out_ext[t, bt * P:(bt + 1) * P, :], in_=cslice)

        nc.compile()
        self.nc = nc
        return nc

    # ---------------- host-side marshalling ----------------
    def marshal_shared(self, fc1_w, fc1_b, fc2_w):
        w1t = np.empty((E, NQ, P, NIC, HID // NQ), dtype=BF)
        w2t = np.empty((E, 2, P, JH, OUT), dtype=BF)
        for e in range(E):
            a = fc1_w[e].T.reshape(NIC, P, HID).transpose(1, 0, 2)
            for q in range(NQ):
                w1t[e, q] = a[:, :, q * (HID // NQ):(q + 1) * (HID // NQ)]
            bm = fc2_w[e].T.reshape(NJT, P, OUT).transpose(1, 0, 2)
            for hh in range(2):
                w2t[e, hh] = bm[:, hh * JH:(hh + 1) * JH, :]
        b1t = np.ascontiguousarray(
            fc1_b.reshape(E, NJT, P).transpose(2, 0, 1)
            .reshape(P, E * NJT)).astype(np.float32)
        return dict(w1t=w1t, w2t=w2t, b1t=b1t)

    def marshal_core(self, c, x, gates, lists):
        """Per-core inputs: compacted x, scatter blocks, gates. Returns
        None if this core's routing does not fit the compiled plan."""
        xt = np.ascontiguousarray(
            x[c * BSH:(c + 1) * BSH].T.reshape(NIC, P, BSH)
            .transpose(1, 0, 2)).astype(BF)       # [P, NIC, BSH]
        idx = np.zeros(self.sumcap, np.int64)
        scat = np.zeros((P, max(self.nblk, 1), P), BF)
        for e in range(E):
            sel = lists[c][e]
            n = len(sel)
            if n > self.ncap[e]:
                return None, None
            idx[self.off[e]:self.off[e] + n] = sel
            for ct in range((n + P - 1) // P):
                sl = sel[ct * P:(ct + 1) * P]
                for s, tok in enumerate(sl):
                    dt, r = divmod(int(tok), P)
                    b = self.blkidx.get((e, ct, dt))
                    if b is None:
                        return None, None
                    scat[s, b, r] = 1
        xcomp = np.ascontiguousarray(xt[:, :, idx])
        gtile = np.ascontiguousarray(
            gates[:, c * BSH:(c + 1) * BSH, :]      # [T, BSH, E]
            .transpose(1, 0, 2).reshape(NBT, P, T * E)
            .transpose(1, 0, 2)).astype(np.float32)
        return dict(xc=xcomp, scat=scat, gt=gtile), True

    def run(self, x, w_gate, fc1_w, fc1_b, fc2_w, fc2_b, _prep=None):
        if _prep is None:
            gates = _route(x, w_gate)
            _, lists, _, _, _ = _plan(gates)
        else:
            gates, lists = _prep
        if self.nc is None:
            self.build()
        shared = self.marshal_shared(fc1_w, fc1_b, fc2_w)
        in_maps = []
        for c in range(NCORES):
            m, ok = self.marshal_core(c, x, gates, lists)
            if not ok:
                return None, None
            m.update(shared)
            in_maps.append(m)
        res = run_bass_kernel_spmd(self.nc, in_maps,
                                   core_ids=list(range(NCORES)))
        out = np.concatenate(
            [res.results[c]["out"] for c in range(NCORES)], axis=1)
        return np.ascontiguousarray(out.astype(np.float32)), res


_KERNEL = None


def kernel(x, w_gate, fc1_w, fc1_b, fc2_w, fc2_b):
    global _KERNEL
    x = np.asarray(x, dtype=np.float32)
    w_gate = np.asarray(w_gate, dtype=np.float32)
    fc1_w = np.asarray(fc1_w, dtype=np.float32)
    fc1_b = np.asarray(fc1_b, dtype=np.float32)
    fc2_w = np.asarray(fc2_w, dtype=np.float32)
    fc2_b = np.asarray(fc2_b, dtype=np.float32)
    assert np.all(fc2_b == 0), "fc2 bias unsupported in sparse path"
    gates = _route(x, w_gate)
    _, lists, ncap, nct, spans = _plan(gates)
    if _KERNEL is None:
        _KERNEL = MMoEKernel(ncap, nct, spans)
    out, _ = _KERNEL.run(x, w_gate, fc1_w, fc1_b, fc2_w, fc2_b,
                         _prep=(gates, lists))
    if out is None:            # routing overflowed the compiled plan
        _KERNEL = MMoEKernel(ncap, nct, spans)
        out, _ = _KERNEL.run(x, w_gate, fc1_w, fc1_b, fc2_w, fc2_b,
                             _prep=(gates, lists))
        assert out is not None
    return out
